# revision 14
# baseline (speedup 1.0000x reference)
import os
os.environ.setdefault('NEURON_RT_DBG_RDH_CC','0')
import numpy as np

N=4096; C=1024; INTER=128; R=128; RR=R*R; GC=256; NCORES=8; NB=N//NCORES
PW=130; HR=R//NCORES           # 16 output h-rows per core
WINR=HR+2                      # 18 padded rows in window
WIN=WINR*PW                    # 2340
QT=(WIN+127)//128              # 19 k-tiles for q
QPAD=QT*128                    # 2432
KT=C//128                      # 8
# split AllGather payloads (elements, bf16)
# AG-A: gx rows 0..255 | b      AG-B: gx rows 256..511 | vraw | zp
GXH=NB*INTER//2                # 32768
AGSA=GXH+NB                    # 33280
VOFFB=GXH; ZOFFB=GXH+INTER; CSOFFB=GXH+INTER+16
AGSB=CSOFFB+C                  # 33936

F32_PARAMS={'sconst','bias128'}

_cache = {}

def _fold(p):
    f32=np.float32
    out={}
    mcw1=p['m_cw'][:INTER]; mcw2=p['m_cw'][INTER:]
    xv=np.zeros((C,6),f32); sc=np.zeros((1,8),f32)
    xv[:,0]=p['m_tw'].T@mcw1; sc[0,0]=p['m_tb']@mcw1            # a
    for j in range(3):
        c1=p['pr_cw'][j,:INTER]; c2=p['pr_cw'][j,INTER:]
        xv[:,1+j]=p['pr_tw'][j].T@c1
        sc[0,1+j]=p['pr_tb'][j]@c1+p['pr_pb'][j]@c2
    xv[:,4]=p['ba_tw'].T@p['ba_cw'][:INTER]
    xv[:,5]=p['m_pw'].T@mcw2; sc[0,5]=p['m_pb']@mcw2            # b
    sc[0,4]=p['ba_tb']@p['ba_cw'][:INTER]+p['ba_pb']@p['ba_cw'][INTER:]
    out['xvecs']=xv; out['sconst']=sc
    vps=np.stack([p['pr_pw'][j].T@p['pr_cw'][j,INTER:] for j in range(3)],1)
    out['vps']=vps.astype(f32)                                   # [C,3]
    out['vpm']=(p['ba_pw'].T@p['ba_cw'][INTER:]/ (2*N)).astype(f32)[:,None]  # [C,1]
    out['m_gwT']=p['m_gw'].T.copy()                              # [C,128]
    out['pr_gwT']=np.stack([p['pr_gw'][j].T for j in range(3)])  # [3,C,128]
    bg=float(p['ba_g'][0])
    out['ba_gwT']=(bg*p['ba_gw'].T/(2*N)).copy()                 # [C,128]
    sg=float(p['sp_g'][0])
    g=np.transpose(p['sp_gw'],(2,3,1,0))[::-1,::-1]              # [kh',kw',ic,oc] reversed
    out['sp_gwT']=np.ascontiguousarray(sg*g.reshape(9*GC,INTER))
    we=np.einsum('c,cikl->ikl',p['sp_cw'][INTER:],p['sp_pw'])    # [GC,3,3]
    out['w_effT']=we.reshape(2,128,9).astype(f32)
    b6=np.zeros((INTER,6),f32)
    b6[:,0]=p['m_gb']; b6[:,1:4]=p['pr_gb'].T; b6[:,4]=bg*p['ba_gb']; b6[:,5]=sg*p['sp_gb']
    out['bias128']=b6
    gf=np.zeros((1,4*INTER),f32)
    for j in range(3): gf[0,j*INTER:(j+1)*INTER]=p['pr_g'][j]
    gf[0,3*INTER:]=1.0
    out['gfill']=gf
    out['mgb_row']=p['m_gb'][None,:].astype(f32)                 # [1,128] K=1 bias trick
    return out

def _shard(p):
    f32=np.float32
    gpadded=np.pad(p['global_feature'][0],((0,0),(1,1),(1,1)))   # [GC,130,130]
    ins=[]
    for k in range(NCORES):
        d={}
        rs=slice(k*NB,(k+1)*NB)
        d['xT']=np.ascontiguousarray(p['origin_feature'][rs].T)
        yt=np.stack([np.ascontiguousarray(t[rs].T) for t in
                     (p['local_feature'],p['bef_l'],p['aft_l'])])
        d['yT']=yt                                               # [3,C,NB]
        d['bafT']=np.ascontiguousarray(np.concatenate(
            [p['bef'][rs],p['aft'][rs]],0).T)                    # [C,2NB]
        gw=gpadded[:,k*HR:k*HR+WINR,:]                           # [GC,18,130]
        d['gpad']=np.ascontiguousarray(gw.reshape(2,128,WINR*PW)
                    .transpose(1,0,2).reshape(128,2*WINR*PW))
        gt=np.zeros((QPAD,GC),f32)
        gt[:WIN]=gw.reshape(GC,WIN).T
        d['gpadT']=gt.reshape(QT,128,GC)
        ins.append(d)
    return ins

def _in_maps(inputs):
    import ml_dtypes
    bf16=ml_dtypes.bfloat16
    fold=_fold(inputs); shards=_shard(inputs)
    in_maps=[]
    for k in range(NCORES):
        m=dict(shards[k]); m.update(fold)
        out={}
        for kk,v in m.items():
            dt=np.float32 if kk in F32_PARAMS else bf16
            out[kk]=np.ascontiguousarray(np.asarray(v,dtype=np.float32).astype(dt))
        in_maps.append(out)
    return in_maps

def kernel(**inputs):
    import ml_dtypes  # noqa
    if 'nc' not in _cache:
        _cache['nc']=build()
    nc=_cache['nc']
    in_maps=_in_maps(inputs)
    from concourse.bass_utils import run_bass_kernel_spmd
    res=run_bass_kernel_spmd(nc,in_maps,list(range(NCORES)))
    _cache['last_res']=res
    out=np.empty((N,INTER),np.float32)
    for k in range(NCORES):
        out[k*NB:(k+1)*NB]=res.results[k]['out'].T
    return out


# ---- device program builder (inlined) ----
import numpy as np
import bass_rust
import concourse.bass as bass
import concourse.bacc as bacc
import concourse.mybir as mybir
import concourse.tile as tile

F32=mybir.dt.float32
BF=mybir.dt.bfloat16
AF=mybir.ActivationFunctionType
AL=mybir.AluOpType
RG=[list(range(NCORES))]

def mkap(a,offset,dims):
    b=a.copy(); b.offset=offset
    b.ap=bass_rust.VecI64Pair([list(d) for d in dims])
    return b

def build():
    nc=bacc.Bacc("TRN2",target_bir_lowering=False,debug=False,num_devices=NCORES)
    P=lambda n,s: nc.declare_dram_parameter(n,list(s),BF,isOutput=False)
    Pf=lambda n,s: nc.declare_dram_parameter(n,list(s),F32,isOutput=False)
    xT=P('xT',(C,NB)); yT=P('yT',(3,C,NB)); bafT=P('bafT',(C,2*NB))
    gpad=P('gpad',(128,2*WIN)); gpadT=P('gpadT',(QT,128,GC))
    xv=P('xvecs',(C,6)); sc=Pf('sconst',(1,8)); vps=P('vps',(C,3)); vpm=P('vpm',(C,1))
    mgw=P('m_gwT',(C,INTER)); prgw=P('pr_gwT',(3,C,INTER)); bagw=P('ba_gwT',(C,INTER))
    spgw=P('sp_gwT',(9*GC,INTER)); weT=P('w_effT',(2,128,9)); b6=Pf('bias128',(INTER,6))
    gf=P('gfill',(1,4*INTER)); mgbr=P('mgb_row',(1,INTER))
    out_ext=nc.declare_dram_parameter('out',[INTER,NB],F32,isOutput=True)
    dbg_ext=nc.declare_dram_parameter('dbg',[128,8],F32,isOutput=True)

    with tile.TileContext(nc) as tc:
      with (tc.tile_pool(name="pp",bufs=1) as pp,
            tc.tile_pool(name="ww",bufs=4) as ww,
            tc.tile_pool(name="dr",bufs=1,space="DRAM") as dr,
            tc.tile_pool(name="ps_or",bufs=1,space="PSUM") as ps_or,
            tc.tile_pool(name="ps_six",bufs=1,space="PSUM") as ps_six,
            tc.tile_pool(name="ps_mid",bufs=2,space="PSUM") as ps_mid,
            tc.tile_pool(name="ps_roll",bufs=2,space="PSUM") as ps_roll,
            tc.tile_pool(name="ps_sm",bufs=1,space="PSUM") as ps_sm):
        dma=nc.sync.dma_start; dmas=nc.scalar.dma_start
        agA_in=dr.tile([AGSA],BF); agA_out=dr.tile([NCORES*AGSA],BF,addr_space='Shared')
        agB_in=dr.tile([AGSB],BF); agB_out=dr.tile([NCORES*AGSB],BF,addr_space='Shared')
        p_loc=dr.tile([2816],BF); q_dr=dr.tile([QPAD],BF)
        zz=pp.tile([128,22],BF,tag='zz'); nc.vector.memset(zz[:],0.0)
        # ---- input DMAs: colsum source first (feeds the first collective) ----
        def ld(name,shape,src_ap):
            t=pp.tile(shape,BF,tag=name); dma(t[:],src_ap); return t
        bft_all=pp.tile([128,KT,2*NB],BF,tag='bft_all')
        dma(bft_all[:],bafT.ap().rearrange("(k p) n -> p k n",p=128))
        gp_s=pp.tile([128,2,WIN],BF,tag='big',name='gp_s',padded_shape=[128,2,WIN])
        dma(gp_s[:],gpad.ap().rearrange("p (h w) -> p h w",h=2))
        gpt_s=pp.tile([128,QT,GC],BF,tag='gpt_s')
        dma(gpt_s[:],gpadT.ap().rearrange("t p g -> p t g"))
        xT_s=ld('xT',[128,KT,NB],xT.ap().rearrange("(k p) n -> p k n",p=128))
        mgw_s=ld('mgw',[128,KT,INTER],mgw.ap().rearrange("(k p) n -> p k n",p=128))
        spg_s=ld('spg',[128,18,INTER],spgw.ap().rearrange("(k p) n -> p k n",p=128))
        we_s=ld('we',[128,2,9],weT.ap().rearrange("h p n -> p h n"))
        xv_s=ld('xv',[128,KT,6],xv.ap().rearrange("(k p) n -> p k n",p=128))
        sc_s=pp.tile([1,8],F32,tag='sc'); dma(sc_s[:],sc.ap())
        b6_s=pp.tile([INTER,6],F32,tag='b6'); dma(b6_s[:],b6.ap())
        gf_s=ld('gf',[1,4*INTER],gf.ap())
        mgbr_s=ld('mgbr',[1,INTER],mgbr.ap())
        ones_c=pp.tile([128,1],F32,tag='ones_c'); nc.vector.memset(ones_c[:],1.0)
        ones_rf=pp.tile([1,128],F32,tag='ones_rf'); nc.vector.memset(ones_rf[:],1.0)
        ONESR=gf_s[0:1,3*INTER:4*INTER]
        # ---- colsums -> first (skew-absorbing) AllGather ----
        cs_f=pp.tile([128,KT],F32,tag='cs_f')
        for kt in range(KT):
            nc.vector.tensor_reduce(cs_f[:,kt:kt+1],bft_all[:,kt,:],
                                    axis=mybir.AxisListType.X,op=AL.add)
        cs_sb=pp.tile([128,KT],BF,tag='cs_sb')
        nc.vector.tensor_copy(cs_sb[:],cs_f[:])
        dmas(agB_in[CSOFFB:CSOFFB+C],cs_sb[:])
        # ---- conv -> b_s own rows ----
        outc=pp.tile([9,WIN],BF,tag='outc')
        for ch in range(5):
            pc=ps_mid.tile([128,512],F32,tag='mid')
            for h in range(2):
                nc.tensor.matmul(pc[:9,:468],we_s[:,h,:],gp_s[:,h,ch*468:(ch+1)*468],
                                 start=(h==0),stop=(h==1))
            nc.scalar.activation(outc[:,ch*468:(ch+1)*468],pc[:9,:468],AF.Copy)
        ov=outc[:].rearrange("p (h w) -> p h w",w=PW)
        bsa=pp.tile([HR,128],BF,tag='bsa')
        for m in range(9):
            kh,kw=divmod(m,3)
            bt=ww.tile([HR,128],BF,tag='bt')
            dmas(bt[:],ov[m:m+1,kh:kh+HR,kw:kw+128])
            if m==0: nc.vector.tensor_copy(bsa[:],bt[:])
            else: nc.vector.tensor_tensor(bsa[:],bsa[:],bt[:],AL.add)
        # ---- local softmax numerator + Z partial ----
        e_own=pp.tile([HR,128],BF,tag='e_own'); ze=pp.tile([HR,1],F32,tag='ze')
        nc.scalar.activation(e_own[:],bsa[:],AF.Exp,accum_out=ze[:])
        pzp=ps_sm.tile([128,512],F32,tag='sm')
        nc.tensor.matmul(pzp[:1,:1],ze[:],ones_c[:HR,:],start=True,stop=True)
        zp_sb=pp.tile([1,1],BF,tag='zp_sb')
        nc.vector.tensor_copy(zp_sb[:],pzp[:1,:1])
        dmas(agB_in[ZOFFB:ZOFFB+1],zp_sb[:])
        dmas(p_loc[:],zz[:,:22])
        dmas(mkap(p_loc[:],262,[(PW,HR),(1,128)]),e_own[:])
        # ---- q correlation (own rows, unnormalized) ----
        lq_all=pp.tile([128,QT,3,3],BF,tag='lq_all')
        for kh in range(3):
            dmas(lq_all[:,:,kh,:],mkap(p_loc[:],130*kh,[(1,128),(128,QT),(1,3)]))
        pq=ps_mid.tile([128,512],F32,tag='mid')
        for t in range(QT):
            nc.tensor.matmul(pq[:9,:GC],lq_all[:,t],gpt_s[:,t,:],start=(t==0),stop=(t==QT-1))
        q_sb=pp.tile([9,GC],BF,tag='q_sb')
        nc.scalar.activation(q_sb[:],pq[:9,:GC],AF.Copy)
        dmas(q_dr[0:9*GC],q_sb[:])
        qd=pp.tile([128,18],BF,tag='qd')
        dmas(qd[:],mkap(q_dr[:],0,[(1,128),(128,18)]))
        pv=ps_sm.tile([128,512],F32,tag='sm2')
        for t in range(18):
            nc.tensor.matmul(pv[:,:1],spg_s[:,t,:],qd[:,t:t+1],start=(t==0),stop=(t==17))
        vr_sb=pp.tile([128,1],BF,tag='vr_sb')
        nc.vector.tensor_copy(vr_sb[:],pv[:,:1])
        dmas(agB_in[VOFFB:VOFFB+INTER],vr_sb[:])
        # ---- psum6: a,b + pair/ba score x-parts ----
        p6=ps_six.tile([6,512],F32,tag='six')
        for kt in range(KT):
            nc.tensor.matmul(p6[:,:],xv_s[:,kt,:],xT_s[:,kt,:],start=(kt==0),
                             stop=(kt==KT-1))
        p6sb=pp.tile([6,512],F32,tag='p6sb')
        nc.scalar.activation(p6sb[:],p6[:,:],AF.Copy)
        p6all=pp.tile([1,6*512],F32,tag='p6all')
        dmas(p6all[:],p6sb[:])
        p6r=[p6all[0:1,512*r:512*(r+1)] for r in range(6)]
        b_sb=pp.tile([1,512],BF,tag='b_sb')
        nc.vector.tensor_scalar(b_sb[:],p6r[5],sc_s[0:1,5:6],None,AL.add)
        dmas(agA_in[GXH:GXH+NB],b_sb[:])
        a_sb=pp.tile([1,512],BF,tag='a_sb')
        nc.vector.tensor_scalar(a_sb[:],p6r[0],sc_s[0:1,0:1],None,AL.add)
        # ---- g_x row-major ----
        gxo=pp.tile([128,4,INTER],BF,tag='gxo')
        for i4 in range(4):
            pg=ps_mid.tile([128,512],F32,tag='mid')
            for kt in range(KT):
                nc.tensor.matmul(pg[:,:INTER],xT_s[:,kt,i4*128:(i4+1)*128],mgw_s[:,kt,:],
                                 start=(kt==0),stop=False,skip_group_check=True)
            nc.tensor.matmul(pg[:,:INTER],ONESR,mgbr_s[:],start=False,stop=True,
                             skip_group_check=True)
            nc.scalar.activation(gxo[:,i4,:],pg[:,:INTER],AF.Copy)
        dmas(mkap(agA_in[:],0,[(128,128),(16384,2),(1,128)]),gxo[:,0:2,:])
        dmas(mkap(agB_in[:],0,[(128,128),(16384,2),(1,128)]),gxo[:,2:4,:])
        # a-broadcast for fT (local; runs during the collectives)
        pab=ps_roll.tile([128,512],F32,tag='roll')
        nc.tensor.matmul(pab[:,:],ONESR,a_sb[:],start=True,stop=True)
        ab_sb=pp.tile([128,512],BF,tag='ab_sb')
        nc.scalar.activation(ab_sb[:],pab[:,:],AF.Copy)
        # ---- the two gx AllGathers ----
        nc.gpsimd.collective_compute("AllGather",AL.bypass,ins=[agA_in[:].opt()],
                                     outs=[agA_out[:].opt()],replica_groups=RG)
        nc.gpsimd.collective_compute("AllGather",AL.bypass,ins=[agB_in[:].opt()],
                                     outs=[agB_out[:].opt()],replica_groups=RG)
        # ---- overlap work: s_sbs + pair terms (no AG dependency) ----
        yT_s=ld('yT',[128,3,KT,NB],yT.ap().rearrange("j (k p) n -> p j k n",p=128))
        vp_s=ld('vp',[128,KT,3],vps.ap().rearrange("(k p) n -> p k n",p=128))
        vpm_s=ld('vpm',[128,KT,1],vpm.ap().rearrange("(k p) n -> p k n",p=128))
        pr_s=ld('pr',[128,3,KT,INTER],prgw.ap().rearrange("j (k p) n -> p j k n",p=128))
        bag_s=ld('bag',[128,KT,INTER],bagw.ap().rearrange("(k p) n -> p k n",p=128))
        s_sbs=[]
        for j in range(3):
            s_sbs.append(pp.tile([1,512],BF,tag=f's_sb{j}',name=f's_sb{j}'))
            psv=ps_mid.tile([128,512],F32,tag='mid')
            for kt in range(KT):
                nc.tensor.matmul(psv[:1,:],vp_s[:,kt,j:j+1],yT_s[:,j,kt,:],
                                 start=(kt==0),stop=(kt==KT-1))
            spre=ww.tile([1,512],F32,tag='spre',bufs=1)
            nc.vector.tensor_scalar(spre[:],psv[:1,:],sc_s[0:1,1+j:2+j],None,AL.add)
            t2=ww.tile([1,512],F32,tag='t2',bufs=1)
            nc.vector.tensor_tensor(t2[:],p6r[1+j],spre[:],AL.add)
            nc.scalar.activation(s_sbs[j][:],t2[:],AF.Relu)
        acc=pp.tile([128,512],F32,tag='acc')
        tmp=pp.tile([128,512],F32,tag='tmp')
        for j in range(3):
            py=ps_roll.tile([128,512],F32,tag='roll')
            for kt in range(KT):
                nc.tensor.matmul(py[:,:],pr_s[:,j,kt,:],yT_s[:,j,kt,:],
                                 start=(kt==0),stop=(kt==KT-1))
            gy=ww.tile([128,512],F32,tag='gy',bufs=1)
            nc.vector.tensor_scalar(gy[:],py[:,:],b6_s[:,1+j:2+j],None,AL.add)
            pb=ps_roll.tile([128,512],F32,tag='roll')
            nc.tensor.matmul(pb[:,:],gf_s[0:1,j*INTER:(j+1)*INTER],s_sbs[j][:],
                             start=True,stop=True)
            if j==0:
                nc.vector.tensor_tensor(acc[:],gy[:],pb[:,:],AL.mult)
            else:
                nc.vector.tensor_tensor(tmp[:],gy[:],pb[:,:],AL.mult)
                nc.vector.tensor_tensor(acc[:],acc[:],tmp[:],AL.add)
        # ---- post-AG-A: gx-A readbacks ----
        gxA_cs=[]
        for c in range(NCORES):
            g=pp.tile([128,2,128],BF,tag=f'gxA{c}',name=f'gxA{c}')
            dma(g[:],mkap(agA_out[:],c*AGSA,[(128,128),(16384,2),(1,128)]))
            gxA_cs.append(g)
        b_allb=pp.tile([128,NCORES,4],BF,tag='b_allb')
        for c in range(NCORES):
            dma(b_allb[:,c,:],mkap(agA_out[:],c*AGSA+GXH,[(1,128),(128,4)]))
        b_allf=pp.tile([128,NCORES,4],F32,tag='b_allf')
        nc.vector.tensor_copy(b_allf[:],b_allb[:])
        # origin first half (gx rows 0..255 of each core) — overlaps AG-B
        po=ps_or.tile([128,512],F32,tag='orig')
        def origin_half(half,gx_cs):
            for k in range(16):
                cc,l=divmod(k,2); lt=half*2+l; jt=half*16+k
                bc=b_allf[:,cc,lt:lt+1]
                fT=ww.tile([128,512],BF,tag='fT',bufs=2)
                if jt%8<3:
                    nc.scalar.activation(fT[:],ab_sb[:],AF.Relu,bias=bc)
                else:
                    nc.vector.tensor_scalar(fT[:],ab_sb[:],bc,0.0,AL.add,AL.max)
                nc.tensor.matmul(po[:,:],gx_cs[cc][:,l,:],fT[:],start=(jt==0),
                                 stop=(jt==31),skip_group_check=True)
        origin_half(0,gxA_cs)
        # ---- post-AG-B: gx-B readbacks, v and Z totals, origin second half ----
        gxB_cs=[]
        for c in range(NCORES):
            g=pp.tile([128,2,128],BF,tag=f'gxB{c}',name=f'gxB{c}')
            dma(g[:],mkap(agB_out[:],c*AGSB,[(128,128),(16384,2),(1,128)]))
            gxB_cs.append(g)
        vall=pp.tile([128,NCORES],BF,tag='vall')
        dma(vall[:],mkap(agB_out[:],VOFFB,[(1,128),(AGSB,NCORES)]))
        zall=pp.tile([1,NCORES],BF,tag='zall')
        dma(zall[:],mkap(agB_out[:],ZOFFB,[(1,1),(AGSB,NCORES)]))
        csall=pp.tile([128,KT,NCORES],BF,tag='csall')
        for c in range(NCORES):
            dma(csall[:,:,c],mkap(agB_out[:],c*AGSB+CSOFFB,[(8,128),(1,8)]))
        z_f=pp.tile([1,1],F32,tag='z_f')
        nc.vector.tensor_reduce(z_f[:],zall[:],axis=mybir.AxisListType.X,op=AL.add)
        zr=pp.tile([1,1],F32,tag='zr'); nc.vector.reciprocal(zr[:],z_f[:])
        pzb=ps_sm.tile([128,512],F32,tag='sm')
        nc.tensor.matmul(pzb[:,:1],ones_rf[:],zr[:],start=True,stop=True)
        zrb=pp.tile([128,1],F32,tag='zrb'); nc.vector.tensor_copy(zrb[:],pzb[:,:1])
        vsum=pp.tile([128,1],F32,tag='vsum')
        nc.vector.tensor_reduce(vsum[:],vall[:],axis=mybir.AxisListType.X,op=AL.add)
        v_sb=pp.tile([128,1],F32,tag='v_sb')
        nc.vector.tensor_scalar(v_sb[:],vsum[:],zrb[:],b6_s[:,5:6],AL.mult,AL.add)
        origin_half(1,gxB_cs)
        # ---- gm/pm from gathered colsums ----
        cs_tf=pp.tile([128,KT],F32,tag='cs_tf')
        for kt in range(KT):
            nc.vector.tensor_reduce(cs_tf[:,kt:kt+1],csall[:,kt,:],
                                    axis=mybir.AxisListType.X,op=AL.add)
        cs_tb=pp.tile([128,KT],BF,tag='cs_tb')
        nc.vector.tensor_copy(cs_tb[:],cs_tf[:])
        pgm=ps_sm.tile([128,512],F32,tag='sm2')
        ppm=ps_sm.tile([128,512],F32,tag='sm')
        for kt in range(KT):
            nc.tensor.matmul(pgm[:,:1],bag_s[:,kt,:],cs_tb[:,kt:kt+1],start=(kt==0),
                             stop=(kt==KT-1),skip_group_check=True)
            nc.tensor.matmul(ppm[:1,:1],vpm_s[:,kt,:],cs_tb[:,kt:kt+1],start=(kt==0),
                             stop=(kt==KT-1),skip_group_check=True)
        gm_sb=pp.tile([128,1],F32,tag='gm_sb')
        nc.vector.tensor_scalar(gm_sb[:],pgm[:,:1],b6_s[:,4:5],None,AL.add)
        pm_sb=pp.tile([1,1],F32,tag='pm_sb')
        nc.vector.tensor_scalar(pm_sb[:],ppm[:1,:1],sc_s[0:1,4:5],None,AL.add)
        sba=pp.tile([1,512],BF,tag='sba')
        nc.scalar.activation(sba[:],p6r[4],AF.Relu,bias=pm_sb[0:1,0:1])
        psb=ps_roll.tile([128,512],F32,tag='roll')
        nc.tensor.matmul(psb[:,:],ONESR,sba[:],start=True,stop=True)
        nc.vector.tensor_scalar(tmp[:],psb[:,:],gm_sb[:],None,AL.mult)
        nc.vector.tensor_tensor(acc[:],acc[:],tmp[:],AL.add)
        ot=pp.tile([128,512],F32,tag='ot')
        nc.vector.tensor_scalar(ot[:],po[:,:],1.0/N,v_sb[:],AL.mult,AL.add)
        fin=pp.tile([128,512],F32,tag='fin')
        nc.vector.tensor_tensor(fin[:],acc[:],ot[:],AL.add)
        dma(out_ext.ap(),fin[:])
        dbt=pp.tile([128,8],F32,tag='dbt')
        nc.vector.tensor_copy(dbt[:,0:1],v_sb[:])
        nc.vector.tensor_copy(dbt[:,1:2],gm_sb[:])
        nc.vector.tensor_copy(dbt[:,2:3],zrb[:])
        nc.vector.tensor_copy(dbt[:,3:4],vsum[:])
        nc.vector.tensor_copy(dbt[:,4:5],acc[:,0:1])
        nc.vector.tensor_copy(dbt[:,5:6],ot[:,0:1])
        nc.vector.tensor_copy(dbt[:,6:7],ab_sb[:,0:1])
        nc.vector.tensor_copy(dbt[:,7:8],cs_tf[:,0:1])
        dma(dbg_ext.ap(),dbt[:])
    nc.compile()
    return nc
